# revision 11
# baseline (speedup 1.0000x reference)
"""Trainium2 Bass kernel for nn_CorrelationLayer (v2 — SBUF-resident inputs).

Reference computation (per sample, C=256, H=W=64, s=8):
    corr  = 0.5*(corr_branch(x0) + corr_branch(x1))        # [64, H, W]
    red   = relu(instance_norm(conv1x1(corr, w_red1)))     # b_red1 cancels in IN
    red   = conv3x3(red, w_red2) + b_red2                  # [256, H, W]
    new   = relu(conv1x1(concat(x0, red), w_adapt) + b_adapt)
    depth = instance_norm(x1)
where corr_branch(x) = l2norm_c(avgpool8(x)) . l2norm_c(x) (cosine maps).

Sharding: pure data parallel, 2 samples per core on 8 cores.

v2 layout: x0/x1 live in SBUF for the whole sample (loaded once, 8 MiB);
phase II reads them from SBUF instead of re-streaming HBM, and the adapt
conv's x0 half is pre-accumulated into `acc0` while x0 is resident, so
phase IV needs no reload.  HBM traffic drops 56 -> ~33 MiB per core.
Intermediates (corr, red1, pad, r2, acc0) are bf16: same PE throughput,
half the SBUF/DVE bandwidth, well inside the 2e-2 error budget.
rsqrt is computed as Exp(-0.5*Ln(x)) on the scalar engine (AF.Rsqrt is
blocked for accuracy).  Sample 1's loads+phase-I compute are woven into
sample 0's conv3x3 window.
"""

import sys

sys.path.insert(0, "/opt/trn_rl_repo")

import numpy as np
import ml_dtypes
from contextlib import ExitStack

import concourse.bass as bass
import concourse.tile as tile
from concourse import bacc, mybir
from concourse.bass_utils import run_bass_kernel_spmd

AF = mybir.ActivationFunctionType
ALU = mybir.AluOpType
AX = mybir.AxisListType
F32 = mybir.dt.float32
F32R = mybir.dt.float32r
BF16 = mybir.dt.bfloat16

N_CORES = 8
B, C, H, W = 16, 256, 64, 64
HW = H * W            # 4096
S2 = 64               # corr_size**2
SPC = B // N_CORES    # samples per core = 2
NCH = 8               # 512-px chunks per image
CHUNK = HW // NCH     # 512
EPS_IN = 1e-5
LN_QUARTER = float(np.log(0.25))

_CACHE = {}


def _r(ap):
    return ap.bitcast(F32R)


def _f(ap):
    return ap.bitcast(F32)


class _Bacc(bacc.Bacc):
    """Bacc whose ACT-table chooser is pinned to the one set that covers
    every function this kernel uses (square/ln/exp/copy/identity/relu).
    The default greedy chooser can alternate between partial sets and
    insert per-chunk table loads at 1283 ns each."""

    _ACT_SET = "natural_log_exp_and_others"

    def insert_act_table_loads(self):
        import concourse.mybir as _mb
        from concourse.hw_specs import get_activation_tables
        import bass_rust as _br
        has_activation = any(
            isinstance(i, _mb.InstActivation)
            for blk in self.main_func.blocks
            for i in blk.instructions
        )
        if not has_activation:
            return
        tables = [
            (name, funcs if name == self._ACT_SET else set())
            for name, funcs in get_activation_tables(self.m.arch).items()
        ]
        _br.insert_act_table_loads(self, tables)


def _build_program():
    nc = _Bacc("TRN2", target_bir_lowering=False, debug=False,
               num_devices=N_CORES)

    x0_d = nc.dram_tensor("x0", [SPC, C, HW], F32, kind="ExternalInput").ap()
    x1_d = nc.dram_tensor("x1", [SPC, C, HW], F32, kind="ExternalInput").ap()
    w1t_d = nc.dram_tensor("w1t", [S2, C], BF16, kind="ExternalInput").ap()
    w2t_d = nc.dram_tensor("w2t", [2, 128, 9 * C], BF16,
                           kind="ExternalInput").ap()
    watr_d = nc.dram_tensor("watr", [128, 2 * C], F32,
                            kind="ExternalInput").ap()
    watb_d = nc.dram_tensor("watb", [128, 2 * C], BF16,
                            kind="ExternalInput").ap()
    b2_d = nc.dram_tensor("b2", [2, 128], F32, kind="ExternalInput").ap()
    ba_d = nc.dram_tensor("ba", [2, 128], F32, kind="ExternalInput").ap()
    nf_d = nc.dram_tensor("nf", [SPC, C, HW], F32, kind="ExternalOutput").ap()
    df_d = nc.dram_tensor("df", [SPC, C, HW], F32, kind="ExternalOutput").ap()

    with tile.TileContext(nc) as tc, ExitStack() as ctx:
        tcp = lambda **kw: ctx.enter_context(tc.tile_pool(**kw))
        p_w = tcp(name="weights", bufs=1)
        p_x = tcp(name="x", bufs=4)          # [128,4096] f32r resident inputs
        p_sq = tcp(name="sq", bufs=2)        # [128,512] squares
        p_p1 = tcp(name="p1", bufs=8)        # [128,512] bf16 pool stage1
        p_ipx = tcp(name="ipx", bufs=16)     # [128,512] bf16 0.5/pixnorm
        p_small = tcp(name="small", bufs=36)  # stats & pooled tiles
        p_tt = tcp(name="tt", bufs=2)        # [64,512] combine tmp
        p_corr = tcp(name="corr", bufs=3)    # [64,512] bf16 corr chunks
        p_rr = tcp(name="rr", bufs=2)        # [128,4096] bf16 red1 pre-IN
        p_pad = tcp(name="pad", bufs=2)      # [128,4356] bf16 conv input
        p_r2 = tcp(name="r2", bufs=2)        # [128,4096] bf16 conv out
        p_acc = tcp(name="acc", bufs=2)      # [128,4096] bf16 adapt-x0 part
        p_df = tcp(name="df", bufs=2)        # [128,1024] f32 depth out
        p_tmp = tcp(name="tmp", bufs=2)      # [128,512] f32 adapt sum
        p_nf = tcp(name="nf", bufs=3)        # [128,512] f32 new_feat out

        ps1 = tcp(name="ps1", bufs=2, space="PSUM")   # phase I cs/nk
        ps2 = tcp(name="ps2", bufs=2, space="PSUM")   # corr cp
        ps3 = tcp(name="ps3", bufs=2, space="PSUM")   # red1/part0/adapt
        ps4 = tcp(name="ps4", bufs=2, space="PSUM")   # conv3x3

        # ---- constants & weights (once) ----
        ones_f = p_w.tile([128, 128], F32)
        nc.vector.memset(ones_f[:], 1.0)
        ones = p_w.tile([128, 128], F32R)
        nc.scalar.copy(ones[:], ones_f[:])
        eps_c = p_w.tile([128, 1], F32)
        nc.vector.memset(eps_c[:], EPS_IN)
        lnq_c = p_w.tile([128, 1], F32)
        nc.vector.memset(lnq_c[:], LN_QUARTER)
        # small weights first (part0 needs watr/ba almost immediately);
        # the big conv weights (w2t/watb) aren't needed until phase III.
        watr_sb = p_w.tile([128, 2 * C], F32R)
        nc.sync.dma_start(watr_sb[:], _r(watr_d[:]))
        w1t_sb = p_w.tile([S2, C], BF16)
        nc.sync.dma_start(w1t_sb[:], w1t_d[:])
        b2_sb = p_w.tile([128, 2], F32)
        ba_sb = p_w.tile([128, 2], F32)
        for mt in range(2):
            nc.sync.dma_start(b2_sb[:, mt:mt + 1], b2_d[mt].unsqueeze(1))
            nc.sync.dma_start(ba_sb[:, mt:mt + 1], ba_d[mt].unsqueeze(1))

        st = [dict() for _ in range(SPC)]
        nonlocal_w2t = [None]
        nonlocal_watb = [None]

        def load_x(s, c2s=(0, 1), quarters=False):
            d = st[s]
            if 'x' not in d:
                d['x'] = [[p_x.tile([128, HW], F32R, name=f"x{i}_{s}_{t}",
                                    tag="x") for t in range(2)]
                          for i in range(2)]
            if quarters:
                # 1024-px pieces, x0/x1 interleaved, so the first part0 and
                # cs matmuls start ~3 us in instead of ~10
                for q in range(4):
                    qsl = slice(q * 1024, (q + 1) * 1024)
                    for i, xd in ((0, x0_d), (1, x1_d)):
                        for t in range(2):
                            nc.sync.dma_start(
                                d['x'][i][t][:, qsl],
                                _r(xd[s, t * 128:(t + 1) * 128, qsl]))
                return
            for c2 in c2s:
                hsl = slice(c2 * 2048, (c2 + 1) * 2048)
                for i, xd in ((0, x0_d), (1, x1_d)):
                    for t in range(2):
                        nc.sync.dma_start(
                            d['x'][i][t][:, hsl],
                            _r(xd[s, t * 128:(t + 1) * 128, hsl]))

        def phase1_head(s):
            d = st[s]
            d['ipx'] = [p_ipx.tile([128, CHUNK], BF16, name=f"ipx_{s}_{ch}",
                                   tag="ipx") for ch in range(NCH)]
            d['p1t'] = [[p_p1.tile([128, CHUNK], BF16, name=f"p1_{s}_{i}_{t}",
                                   tag="p1") for t in range(2)]
                        for i in range(2)]
            d['sscols'] = [p_small.tile([128, NCH], F32, name=f"ss1_{s}_{t}",
                                        tag="small") for t in range(2)]
            d['acc0'] = [p_acc.tile([128, HW], BF16, name=f"acc0_{s}_{mt}",
                                    tag="acc") for mt in range(2)]

        def phase1_chunk(s, ch, part0=True):
            d = st[s]
            x = d['x']
            ipx, p1t, sscols = d['ipx'], d['p1t'], d['sscols']
            sl = slice(ch * CHUNK, (ch + 1) * CHUNK)
            csp = [ps1.tile([S2, CHUNK], F32, name=f"cs{i}_{s}_{ch}",
                            tag="ps1") for i in range(2)]
            for i in range(2):
                for t in range(2):
                    xc = x[i][t][:, sl]
                    # pooled stage 1: sum over w within groups of 8
                    # (8-element sums; bf16 output costs ~0.4% on the pooled
                    # features, which are l2-normalized right after)
                    with nc.allow_low_precision(reason="8-el partial pools"):
                        nc.vector.tensor_reduce(
                            p1t[i][t][:, ch * 64:(ch + 1) * 64],
                            _f(xc).rearrange("p (g w) -> p g w", w=8),
                            AX.X, ALU.add)
                    sqc = p_sq.tile([128, CHUNK], F32R,
                                    name=f"sq_{s}_{i}_{t}_{ch}", tag="sq")
                    if i == 0:
                        nc.scalar.square(sqc[:], _f(xc))
                    else:
                        # square + channel sum-of-squares in one DVE op
                        nc.vector.scalar_tensor_tensor(
                            sqc[:], _f(xc), 1.0, _f(xc), ALU.mult, ALU.mult,
                            accum_out=sscols[t][:, ch:ch + 1])
                    nc.tensor.matmul(csp[i][:],
                                     _r(ones[:, :S2]), _r(sqc[:]),
                                     start=(t == 0), stop=(t == 1))
            for i in range(2):
                # 0.5 / ||px|| = exp(-0.5 * (ln(sumsq) - ln(0.25)))
                half = ipx[ch][i * S2:(i + 1) * S2, :]
                nc.scalar.activation(half, csp[i][:], AF.Ln)
                nc.scalar.activation(half, half, AF.Exp,
                                     bias=lnq_c[0:S2, :], scale=-0.5)
            if part0:
                part0_chunk(s, ch)

        def part0_chunk(s, ch):
            # adapt conv, x0 half (kt 0,1) while x0 is resident
            d = st[s]
            x = d['x']
            sl = slice(ch * CHUNK, (ch + 1) * CHUNK)
            for mt in range(2):
                p0 = ps3.tile([128, CHUNK], F32, name=f"p0_{s}_{ch}_{mt}",
                              tag="ps3")
                for t in range(2):
                    lhs = watr_sb[:, t * C + mt * 128:t * C + mt * 128 + 128]
                    nc.tensor.matmul(p0[:], _r(lhs), x[0][t][:, sl],
                                     start=(t == 0), stop=(t == 1))
                # fold b_adapt in here so phase IV is just add+relu
                nc.scalar.activation(d['acc0'][mt][:, sl], p0[:],
                                     AF.Identity, bias=ba_sb[:, mt:mt + 1])

        def phase1_tail(s):
            d = st[s]
            p1t, sscols = d['p1t'], d['sscols']
            khat = []
            for i in range(2):
                kh_t = []
                for t in range(2):
                    pk = p_small.tile([128, S2], F32,
                                      name=f"pooled_{s}_{i}_{t}", tag="small")
                    # p1 index = 64*a + 8*r + w  ->  sum over r
                    nc.vector.tensor_reduce(
                        pk[:],
                        p1t[i][t][:].rearrange("p (a r w) -> p a w r",
                                               a=8, r=8),
                        AX.X, ALU.add)
                    kh_t.append(pk)
                nkp = ps1.tile([128, S2], F32, name=f"nk_{s}_{i}", tag="ps1")
                psq = []
                for t in range(2):
                    pq = p_small.tile([128, S2], F32R, name=f"psq_{s}_{i}_{t}",
                                      tag="small")
                    nc.vector.tensor_tensor(pq[:], kh_t[t][:], kh_t[t][:],
                                            ALU.mult)
                    psq.append(pq)
                for t in range(2):
                    nc.tensor.matmul(nkp[:], _r(ones[:]), _r(psq[t][:]),
                                     start=(t == 0), stop=(t == 1))
                nk_sb = p_small.tile([128, S2], F32, name=f"nk_sb_{s}_{i}",
                                     tag="small")
                nc.scalar.activation(nk_sb[:], nkp[:], AF.Ln)
                invk = p_small.tile([128, S2], F32, name=f"invk_{s}_{i}",
                                    tag="small")
                nc.scalar.activation(invk[:], nk_sb[:], AF.Exp, scale=-0.5)
                kh = []
                for t in range(2):
                    k2 = p_small.tile([128, S2], F32R, name=f"khat_{s}_{i}_{t}",
                                      tag="small")
                    nc.vector.tensor_tensor(k2[:], kh_t[t][:], invk[:],
                                            ALU.mult)
                    kh.append(k2)
                khat.append(kh)

            # depth-feat (instance norm of x1): var = E[x^2] - E[x]^2
            istd1, bneg1 = [], []
            for t in range(2):
                mv = p_small.tile([128, 2], F32, name=f"mv1_{s}_{t}",
                                  tag="small")
                nc.vector.tensor_reduce(mv[:, 0:1], sscols[t][:], AX.X,
                                        ALU.add)
                nc.vector.tensor_reduce(mv[:, 1:2], p1t[1][t][:], AX.X,
                                        ALU.add)
                mvn = p_small.tile([128, 2], F32, name=f"mvn_{s}_{t}",
                                   tag="small")
                nc.vector.tensor_scalar(mvn[:], mv[:], 1.0 / HW, None,
                                        ALU.mult)
                msq = p_small.tile([128, 1], F32, name=f"msq_{s}_{t}",
                                   tag="small")
                nc.vector.tensor_tensor(msq[:], mvn[:, 1:2], mvn[:, 1:2],
                                        ALU.mult)
                var = p_small.tile([128, 1], F32, name=f"var1_{s}_{t}",
                                   tag="small")
                nc.vector.tensor_tensor(var[:], mvn[:, 0:1], msq[:],
                                        ALU.subtract)
                std = p_small.tile([128, 1], F32, name=f"std1_{s}_{t}",
                                   tag="small")
                nc.scalar.activation(std[:], var[:], AF.Ln, bias=eps_c[:])
                ist = p_small.tile([128, 1], F32, name=f"istd1_{s}_{t}",
                                   tag="small")
                nc.scalar.activation(ist[:], std[:], AF.Exp, scale=-0.5)
                bn = p_small.tile([128, 1], F32, name=f"bneg1_{s}_{t}",
                                  tag="small")
                nc.vector.scalar_tensor_tensor(bn[:], mvn[:, 1:2], -1.0,
                                               ist[:], ALU.mult, ALU.mult)
                istd1.append(ist)
                bneg1.append(bn)

            d['khat'] = khat
            d['istd1'] = istd1
            d['bneg1'] = bneg1

        def depth_out(s):
            d = st[s]
            x1, istd1, bneg1 = d['x'][1], d['istd1'], d['bneg1']
            for t in range(2):
                for h in range(4):
                    hsl = slice(h * 1024, (h + 1) * 1024)
                    dfc = p_df.tile([128, 1024], F32,
                                    name=f"dfc_{s}_{t}_{h}", tag="df")
                    nc.vector.tensor_scalar(dfc[:], _f(x1[t][:, hsl]),
                                            istd1[t][:], bneg1[t][:],
                                            ALU.mult, ALU.add)
                    nc.gpsimd.dma_start(
                        df_d[s, t * 128:(t + 1) * 128, hsl], dfc[:])

        def phase2_head(s):
            d = st[s]
            d['rr'] = [p_rr.tile([128, HW], BF16, name=f"rr_{s}_{mt}",
                                 tag="rr") for mt in range(2)]
            d['bnsr'] = [p_small.tile([128, NCH * 6], F32,
                                      name=f"bnsr_{s}_{mt}", tag="small")
                         for mt in range(2)]

        def phase2_chunk(s, ch):
            d = st[s]
            x, ipx, khat = d['x'], d['ipx'], d['khat']
            rr, bnsr = d['rr'], d['bnsr']
            sl = slice(ch * CHUNK, (ch + 1) * CHUNK)
            cp = [ps2.tile([S2, CHUNK], F32, name=f"c{i}_{s}_{ch}",
                           tag="ps2") for i in range(2)]
            for i in range(2):
                for t in range(2):
                    nc.tensor.matmul(cp[i][:], _r(khat[i][t][:]),
                                     x[i][t][:, sl],
                                     start=(t == 0), stop=(t == 1))
            tts = []
            for i in range(2):
                t_ = p_tt.tile([S2, CHUNK], BF16, name=f"tt{i}_{s}_{ch}",
                               tag="tt")
                nc.vector.tensor_tensor(t_[:], cp[i][:],
                                        ipx[ch][i * S2:(i + 1) * S2, :],
                                        ALU.mult)
                tts.append(t_)
            corr_c = p_corr.tile([S2, CHUNK], BF16, name=f"corr_{s}_{ch}",
                                 tag="corr")
            nc.gpsimd.tensor_tensor(corr_c[:], tts[0][:], tts[1][:], ALU.add)
            # red1 = w1 @ corr  (K = 64)
            for mt in range(2):
                rrp = ps3.tile([128, CHUNK], F32, name=f"rrp_{s}_{ch}_{mt}",
                               tag="ps3")
                nc.tensor.matmul(rrp[:], w1t_sb[:, mt * 128:(mt + 1) * 128],
                                 corr_c[:], start=True, stop=True)
                nc.scalar.copy(rr[mt][:, sl], rrp[:])
                nc.vector.bn_stats(bnsr[mt][:, ch * 6:(ch + 1) * 6],
                                   rr[mt][:, sl])

        def phase2_tail(s):
            d = st[s]
            rr, bnsr = d['rr'], d['bnsr']
            ists, bns = [], []
            for mt in range(2):
                mv = p_small.tile([128, 2], F32, name=f"mvr_{s}_{mt}",
                                  tag="small")
                nc.vector.bn_aggr(mv[:], bnsr[mt][:])
                std = p_small.tile([128, 1], F32, name=f"stdr_{s}_{mt}",
                                   tag="small")
                nc.scalar.activation(std[:], mv[:, 1:2], AF.Ln, bias=eps_c[:])
                ist = p_small.tile([128, 1], F32, name=f"istdr_{s}_{mt}",
                                   tag="small")
                nc.scalar.activation(ist[:], std[:], AF.Exp, scale=-0.5)
                bn = p_small.tile([128, 1], F32, name=f"bnegr_{s}_{mt}",
                                  tag="small")
                nc.vector.scalar_tensor_tensor(bn[:], mv[:, 0:1], -1.0, ist[:],
                                               ALU.mult, ALU.mult)
                ists.append(ist)
                bns.append(bn)
            red_pad = []
            for mt in range(2):
                ist, bn = ists[mt], bns[mt]
                pad = p_pad.tile([128, 66 * 66], BF16, name=f"pad_{s}_{mt}",
                                 tag="pad")
                pv = pad[:].rearrange("p (h w) -> p h w", w=66)
                for brd in (pv[:, 0:1, :], pv[:, 65:66, :],
                            pv[:, 1:65, 0:1], pv[:, 1:65, 65:66]):
                    nc.vector.memset(brd, 0.0)
                rrv = rr[mt][:].rearrange("p (h w) -> p h w", w=64)
                ctr = pv[:, 1:65, 1:65]
                if mt == 0:
                    # ACT path, split so conv3x3's first chunks start before
                    # the whole image is normalized
                    for h0, h1 in ((0, 32), (32, 64)):
                        nc.scalar.activation(
                            ctr[:, h0:h1, :], rrv[:, h0:h1, :],
                            AF.Relu, bias=bn[:], scale=ist[:])
                else:
                    # DVE path: (rr*istd + bneg) then relu via max(.,0),
                    # in place -- keeps the pad build off the ACT critical
                    # path (both engines build their half concurrently)
                    with nc.allow_low_precision(reason="IN scale in bf16"):
                        nc.vector.tensor_scalar(ctr, rrv, ist[:], bn[:],
                                                ALU.mult, ALU.add)
                        nc.vector.tensor_scalar(ctr, ctr, 0.0, None, ALU.max)
                red_pad.append(pad)
            d['red_pad'] = red_pad

        def phase3(s, weave=None, post=None):
            d = st[s]
            red_pad = d['red_pad']
            r2 = [p_r2.tile([128, HW], BF16, name=f"r2_{s}_{mt}", tag="r2")
                  for mt in range(2)]
            d['r2'] = r2
            pvs = [red_pad[kt][:].rearrange("p (h w) -> p h w", w=66)
                   for kt in range(2)]
            for ch in range(NCH):               # 512-px chunks (8 rows)
                if weave is not None:
                    weave(ch)
                for mt in range(2):
                    c3p = ps4.tile([128, CHUNK], F32,
                                   name=f"c3_{s}_{mt}_{ch}", tag="ps4")
                    y0 = ch * 8
                    first = True
                    for off in range(9):
                        dy, dx = off // 3, off % 3
                        for kt in range(2):
                            lhs = nonlocal_w2t[0][kt][:, off * C + mt * 128:
                                                      off * C + mt * 128 + 128]
                            rhs = pvs[kt][:, y0 + dy:y0 + dy + 8, dx:dx + 64]
                            nc.tensor.matmul(
                                c3p[:], lhs, rhs,
                                start=first, stop=(off == 8 and kt == 1))
                            first = False
                    nc.scalar.activation(
                        r2[mt][:, ch * CHUNK:(ch + 1) * CHUNK],
                        c3p[:], AF.Identity, bias=b2_sb[:, mt:mt + 1])
                if post is not None:
                    post(ch)

        def phase4_unit(s, mt, ch):
            d = st[s]
            r2, acc0 = d['r2'], d['acc0']
            sl = slice(ch * CHUNK, (ch + 1) * CHUNK)
            ap_ = ps3.tile([128, CHUNK], F32, name=f"aps_{s}_{mt}_{ch}",
                           tag="ps3")
            for kt in range(2):
                lhs = nonlocal_watb[0][:, kt * C + mt * 128:
                                       kt * C + mt * 128 + 128]
                nc.tensor.matmul(ap_[:], lhs, r2[kt][:, sl],
                                 start=(kt == 0), stop=(kt == 1))
            tmp = p_tmp.tile([128, CHUNK], F32, name=f"tmp_{s}_{mt}_{ch}",
                             tag="tmp")
            nc.vector.tensor_tensor(tmp[:], ap_[:], acc0[mt][:, sl], ALU.add)
            nfc = p_nf.tile([128, CHUNK], F32, name=f"nf_{s}_{mt}_{ch}",
                            tag="nf")
            nc.scalar.activation(nfc[:], tmp[:], AF.Relu)
            nc.gpsimd.dma_start(nf_d[s, mt * 128:(mt + 1) * 128, sl], nfc[:])

        # ================= schedule =================
        load_x(0, quarters=True)
        # big conv weights flow behind x(0); first needed in phase III.
        w2t_sb = []
        for kt in range(2):
            w2 = p_w.tile([128, 9 * C], BF16, name=f"w2t_sb{kt}")
            nc.sync.dma_start(w2[:], w2t_d[kt])
            w2t_sb.append(w2)
        nonlocal_w2t[0] = w2t_sb
        watb_sb = p_w.tile([128, 2 * C], BF16)
        nc.sync.dma_start(watb_sb[:], watb_d[:])
        nonlocal_watb[0] = watb_sb
        phase1_head(0)
        for c in range(4):
            part0_chunk(0, c)
        for ch in range(NCH):
            phase1_chunk(0, ch, part0=(ch >= 4))
        phase1_tail(0)
        phase2_head(0)
        for ch in range(NCH):
            phase2_chunk(0, ch)
        depth_out(0)               # overlaps phase III on DVE/Pool
        phase2_tail(0)

        # sample 1 loads flow during sample 0's conv window; its phase-I
        # compute is woven into the second half of that window (data has
        # landed by then), and sample 0's adapt conv (phase IV) chunks are
        # emitted right after their r2 chunk exists.
        load_x(1)
        phase1_head(1)

        def _weave0(ch):
            if ch >= 4:
                phase1_chunk(1, 2 * (ch - 4), part0=False)
                phase1_chunk(1, 2 * (ch - 4) + 1, part0=False)

        def _post0(ch):
            for mt in range(2):
                phase4_unit(0, mt, ch)

        phase3(0, weave=_weave0, post=_post0)
        phase1_tail(1)
        for ch in range(NCH):
            part0_chunk(1, ch)
        phase2_head(1)
        for ch in range(NCH):
            phase2_chunk(1, ch)
        depth_out(1)
        phase2_tail(1)

        def _post1(ch):
            for mt in range(2):
                phase4_unit(1, mt, ch)

        phase3(1, post=_post1)

    nc.compile()
    return nc


def _get_program():
    if "nc" not in _CACHE:
        _CACHE["nc"] = _build_program()
    return _CACHE["nc"]


def _prep_weights(w_red1, w_red2, w_adapt, b_red2, b_adapt):
    bf = ml_dtypes.bfloat16
    w1t = np.ascontiguousarray(w_red1[:, :, 0, 0].T).astype(bf)    # [64,256]
    w2 = w_red2.transpose(2, 3, 1, 0).reshape(9, C, C)             # off,ci,co
    w2t = np.ascontiguousarray(
        w2.reshape(9, 2, 128, C).transpose(1, 2, 0, 3)
        .reshape(2, 128, 9 * C)).astype(bf)
    wat = np.ascontiguousarray(
        w_adapt[:, :, 0, 0].T.reshape(4, 128, C))                  # kt,ci,co
    watr = np.ascontiguousarray(
        wat[0:2].transpose(1, 0, 2).reshape(128, 2 * C))           # x0 half
    watb = np.ascontiguousarray(
        wat[2:4].transpose(1, 0, 2).reshape(128, 2 * C)).astype(bf)  # r2 half
    b2 = np.ascontiguousarray(b_red2.reshape(2, 128))
    ba = np.ascontiguousarray(b_adapt.reshape(2, 128))
    return w1t, w2t, watr, watb, b2, ba


def make_in_maps(x0, x1, w_red1, b_red1, w_red2, b_red2, w_adapt, b_adapt):
    w1t, w2t, watr, watb, b2, ba = _prep_weights(
        np.asarray(w_red1, np.float32), np.asarray(w_red2, np.float32),
        np.asarray(w_adapt, np.float32), np.asarray(b_red2, np.float32),
        np.asarray(b_adapt, np.float32))
    x0 = np.asarray(x0, np.float32).reshape(B, C, HW)
    x1 = np.asarray(x1, np.float32).reshape(B, C, HW)
    in_maps = []
    for i in range(N_CORES):
        sl = slice(i * SPC, (i + 1) * SPC)
        in_maps.append({
            "x0": np.ascontiguousarray(x0[sl]),
            "x1": np.ascontiguousarray(x1[sl]),
            "w1t": w1t, "w2t": w2t, "watr": watr, "watb": watb,
            "b2": b2, "ba": ba,
        })
    return in_maps


def kernel(x0, x1, w_red1, b_red1, w_red2, b_red2, w_adapt, b_adapt):
    nc = _get_program()
    in_maps = make_in_maps(x0, x1, w_red1, b_red1, w_red2, b_red2,
                           w_adapt, b_adapt)
    res = run_bass_kernel_spmd(nc, in_maps, list(range(N_CORES)))
    nf = np.concatenate([res.results[i]["nf"] for i in range(N_CORES)], axis=0)
    df = np.concatenate([res.results[i]["df"] for i in range(N_CORES)], axis=0)
    return (nf.reshape(B, C, H, W).astype(np.float32),
            df.reshape(B, C, H, W).astype(np.float32))
